# revision 7
# baseline (speedup 1.0000x reference)
"""Trainium2 Bass kernel for nn_HcPost:

    out[b,s,n,d] = post[b,s,n] * x[b,s,d] + sum_m comb[b,s,m,n] * residual[b,s,m,d]

Strategy: per token this is a tiny K=5 contraction
    out[n,d] = sum_{m'} Caug[m',n] * Xaug[m',d]
with Xaug = [x; residual_0..3] and Caug = [post; comb_0..3].

We batch G=25 tokens into one TensorE matmul by building a block-diagonal
stationary weight matrix W[(t,m'), (t,n)] = Caug[t,m',n] (K=125, MF=100) on the
host, and streaming Xaug[(t,m'), d] as the moving operand. PSUM results
[(t,n), d] are evacuated to SBUF by VectorE/ScalarE and DMA'd out.

Sharding: tokens (B*S = 16384) split evenly across 8 NeuronCores (data
parallel, no cross-core communication). Tokens are padded to 2050/core so each
core runs 82 uniform groups of 25.
"""

import sys

sys.path.insert(0, "/opt/trn_rl_repo")

import numpy as np

import concourse.bass as bass
import concourse.mybir as mybir
import concourse.tile as tile
from concourse import bacc
from concourse.bass_utils import run_bass_kernel_spmd

B, S, M, N, D = 4, 4096, 4, 4, 2048
TOK = B * S  # 16384 tokens
N_CORES = 8
G = 25  # tokens per PE group (contraction K = 5*G = 125 <= 128)
KDIM = 5 * G  # 125
MF = N * G  # 100 output partitions per group
TPC = 2050  # padded tokens per core (= 82 * 25)
NG = TPC // G  # 82 groups per core
TOKP = TPC * N_CORES  # 16400 padded tokens total
GP = 4  # groups per DMA chunk (batches DMAs to ~4 MB)
DCH = 512  # moving free-dim chunk (fp32 matmul max / one PSUM bank)

_CHUNKS = []
_g = 0
while _g < NG:
    _CHUNKS.append((_g, min(GP, NG - _g)))
    _g += _CHUNKS[-1][1]

# Stashed BassKernelResults of the last kernel() call (for profiling in test
# harnesses via BASS_TRACE=1).
LAST_RESULTS = None
LAST_IN_MAPS = None

# Best configuration found on HW (564 us/core vs 992 us for the naive
# sync-HWDGE version): all bulk DMAs via gpsimd SWDGE (spreads descriptors
# over all 16 SDMA engines; HWDGE pins reads to engines 0-4 / writes to
# 0-9), single-group chunks with deep double-buffering, output DMAs delayed
# by several chunks so they never head-of-line block input DMAs in the Pool
# FIFO, weight slices interleaved into the first chunks.
#
# fp16 I/O: inputs are converted to fp16 on the host and the output is
# written back as fp16 (upcast on the host), halving HBM traffic. The PE
# multiplies fp16 exactly into fp32 PSUM, so only input rounding
# (2^-11 relative) and the final fp16 output rounding contribute error
# (~3e-4 rel vs the fp32 reference).
BUILD_KWARGS = dict(
    in_eng="gpsimd",
    gp=1,
    abufs=10,
    obufs=9,
    out_spart=100,
    out_delay=6,
    wsplit=8,
    weng="ginter",
    mm_dtype="float16",
    out_dtype="float16",
)


def _build_program(in_eng="sync", out_eng="sync", in_split=1, out_split=1,
                   out_hpart=0, gp=GP, abufs=2, obufs=2, pbufs=8,
                   out_spart=0, out_delay=4, wsplit=1, weng="sync", mm_dtype="float32",
                   out_dtype="float32", copy_banks=1):
    """Build the SPMD Bass program.

    in_eng/out_eng: comma-separated engine cycle for input/output DMAs —
    elements from {sync, scalar, gpsimd}. Successive chunks rotate through
    the cycle. in_split/out_split: issue each chunk's DMA as this many
    instructions (split along the partition dim). out_hpart: if >0, rows
    [0, out_hpart) of each output chunk go via sync HWDGE and the rest via
    gpsimd SWDGE (overrides out_eng).
    """
    f32 = mybir.dt.float32
    mmdt = getattr(mybir.dt, mm_dtype)
    outdt = getattr(mybir.dt, out_dtype)
    nc = bacc.Bacc(None, target_bir_lowering=False)
    xa = nc.dram_tensor("xa", [TPC * 5, D], mmdt, kind="ExternalInput")
    wb = nc.dram_tensor("wb", [KDIM, NG * MF], mmdt, kind="ExternalInput")
    y = nc.dram_tensor("y", [TPC * N, D], outdt, kind="ExternalOutput")

    def engines(spec):
        return [getattr(nc, e) for e in spec.split(",")]

    in_engs = engines(in_eng)
    out_engs = engines(out_eng)

    chunks = []
    g = 0
    while g < NG:
        chunks.append((g, min(gp, NG - g)))
        g += chunks[-1][1]

    # Row r = t*5 + m' of xa is one (token, m') slice; groups are 125 rows.
    xa_v = xa[:].rearrange("(G p) d -> G p d", p=KDIM)
    # Row r = t*4 + n of y; groups are 100 rows.
    y_v = y[:].rearrange("(G p) d -> G p d", p=MF)

    def split_dma(eng, dst, src, nsplit, pdim):
        if nsplit == 1:
            eng.dma_start(dst, src)
            return
        step = (pdim + nsplit - 1) // nsplit
        for s0 in range(0, pdim, step):
            s1 = min(s0 + step, pdim)
            eng.dma_start(dst[s0:s1], src[s0:s1])

    with tile.TileContext(nc) as tc:
        with (
            tc.tile_pool(name="wpool", bufs=1) as wpool,
            tc.tile_pool(name="apool", bufs=abufs) as apool,
            tc.tile_pool(name="opool", bufs=obufs) as opool,
            tc.tile_pool(name="psum", bufs=pbufs, space=bass.MemorySpace.PSUM) as psum,
        ):
            gper = (NG + wsplit - 1) // wsplit
            interleave_w = weng == "ginter"
            wt_tiles = []
            w_eng = nc.gpsimd if (interleave_w or mmdt != f32) else getattr(nc, weng)

            def load_w(wi):
                glo = wi * gper
                ghi = min(NG, (wi + 1) * gper)
                wtile = wpool.tile([KDIM, (ghi - glo) * MF], mmdt, tag=f"w{wi}")
                w_eng.dma_start(wtile[:], wb[:, glo * MF : ghi * MF])
                wt_tiles.append(wtile)

            if not interleave_w:
                for wi in range(wsplit):
                    load_w(wi)

            def w_slice(g):
                wi, off = divmod(g, gper)
                return wt_tiles[wi][:, off * MF : (off + 1) * MF]

            k = 0
            pending = []  # delayed SWDGE output DMAs: (dst_ap, src_tile_ap)
            for ci, (gstart, cgp) in enumerate(chunks):
                a = apool.tile([KDIM, cgp, D], mmdt, tag="a")
                split_dma(
                    in_engs[ci % len(in_engs)],
                    a[:],
                    xa_v[gstart : gstart + cgp].rearrange("g p d -> p g d"),
                    in_split,
                    KDIM,
                )
                if interleave_w and ci < wsplit:
                    load_w(ci)
                if out_spart > 0 and len(pending) >= out_delay:
                    dst, src = pending.pop(0)
                    nc.gpsimd.dma_start(dst, src)
                o = opool.tile([MF, cgp, D], outdt, tag="o")
                for gs in range(cgp):
                    gw = gstart + gs
                    for dcb in range(0, D // DCH, copy_banks):
                        p = psum.tile([MF, copy_banks * DCH], f32)
                        for j in range(copy_banks):
                            dc = dcb + j
                            nc.tensor.matmul(
                                p[:, j * DCH : (j + 1) * DCH],
                                lhsT=w_slice(gw),
                                rhs=a[:, gs, dc * DCH : (dc + 1) * DCH],
                                start=True,
                                stop=True,
                            )
                        dst = o[:, gs, dcb * DCH : (dcb + copy_banks) * DCH]
                        if k % 2 == 0:
                            nc.vector.tensor_copy(dst, p[:])
                        else:
                            nc.scalar.copy(dst, p[:])
                        k += 1
                y_dst = y_v[gstart : gstart + cgp].rearrange("g p d -> p g d")
                if out_spart > 0:
                    hp = MF - out_spart
                    if hp > 0:
                        nc.sync.dma_start(y_dst[:hp], o[:hp])
                    pending.append((y_dst[hp:], o[hp:]))
                elif out_hpart > 0:
                    nc.sync.dma_start(y_dst[:out_hpart], o[:out_hpart])
                    nc.gpsimd.dma_start(y_dst[out_hpart:], o[out_hpart:])
                else:
                    split_dma(
                        out_engs[ci % len(out_engs)],
                        y_dst,
                        o[:],
                        out_split,
                        MF,
                    )
            for dst, src in pending:
                nc.gpsimd.dma_start(dst, src)
    nc.compile()
    return nc


def kernel(x, residual, post, comb):
    global LAST_RESULTS, LAST_IN_MAPS
    x = np.asarray(x, dtype=np.float32)
    residual = np.asarray(residual, dtype=np.float32)
    post = np.asarray(post, dtype=np.float32)
    comb = np.asarray(comb, dtype=np.float32)

    mmdt = np.dtype(BUILD_KWARGS.get("mm_dtype", "float32"))
    outdt = np.dtype(BUILD_KWARGS.get("out_dtype", "float32"))

    # Host prepack: augmented data rows (token-major) and block-diagonal
    # weights. Padded tokens have zero weights -> zero output rows.
    xaug = np.zeros((TOKP, 5, D), mmdt)
    xaug[:TOK, 0, :] = x.reshape(TOK, D)
    xaug[:TOK, 1:, :] = residual.reshape(TOK, M, D)

    caug = np.zeros((TOKP, 5, N), mmdt)
    caug[:TOK, 0, :] = post.reshape(TOK, N)
    caug[:TOK, 1:, :] = comb.reshape(TOK, M, N)

    ngt = TOKP // G  # total groups
    wall = np.zeros((ngt, KDIM, MF), mmdt)
    t = np.arange(G)
    rows = np.broadcast_to(
        5 * t[:, None, None] + np.arange(5)[None, :, None], (G, 5, N)
    ).ravel()
    cols = np.broadcast_to(
        N * t[:, None, None] + np.arange(N)[None, None, :], (G, 5, N)
    ).ravel()
    wall[:, rows, cols] = caug.reshape(ngt, G * 5 * N)

    in_maps = []
    for c in range(N_CORES):
        xa_c = np.ascontiguousarray(xaug[c * TPC : (c + 1) * TPC].reshape(TPC * 5, D))
        wb_c = np.ascontiguousarray(
            wall[c * NG : (c + 1) * NG].transpose(1, 0, 2).reshape(KDIM, NG * MF)
        )
        in_maps.append({"xa": xa_c, "wb": wb_c})

    LAST_IN_MAPS = in_maps
    nc = _build_program(**BUILD_KWARGS)
    res = run_bass_kernel_spmd(nc, in_maps, list(range(N_CORES)))
    LAST_RESULTS = res

    y = np.concatenate(
        [
            res.results[c]["y"].reshape(TPC, N, D).astype(np.float32)
            for c in range(N_CORES)
        ],
        axis=0,
    )[:TOK]
    return np.ascontiguousarray(y.reshape(B, S, N, D))



# revision 19
# speedup vs baseline: 1.0361x; 1.0361x over previous
"""Trainium2 Bass kernel for nn_HcPost:

    out[b,s,n,d] = post[b,s,n] * x[b,s,d] + sum_m comb[b,s,m,n] * residual[b,s,m,d]

Strategy: per token this is a tiny K=5 contraction
    out[n,d] = sum_{m'} Caug[m',n] * Xaug[m',d]
with Xaug = [x; residual_0..3] and Caug = [post; comb_0..3].

We batch G=25 tokens into one TensorE matmul by building a block-diagonal
stationary weight matrix W[(t,m'), (t,n)] = Caug[t,m',n] (K=125, MF=100) on the
host, and streaming Xaug[(t,m'), d] as the moving operand. PSUM results
[(t,n), d] are evacuated to SBUF by VectorE/ScalarE and DMA'd out.

Sharding: tokens (B*S = 16384) split evenly across 8 NeuronCores (data
parallel, no cross-core communication). Tokens are padded to 2050/core so each
core runs 82 uniform groups of 25.

I/O layout: everything is fp16 (inputs converted on the host, output upcast on
the host; PE multiplies fp16 exactly into fp32 PSUM, so the only error is the
2^-11 input/output rounding, ~6e-4 rel vs the fp32 reference) and
PARTITION-MAJOR in HBM: xa[p, g*D+d] for p = 5*t_loc + m', y[q, g*D+d] for
q = 4*t_loc + n. That makes each DMA chunk of `gp` groups a single op whose
per-partition runs are gp*4KB contiguous, so SWDGE emits few, large
descriptors (vs one 4KB descriptor per partition in token-major layout),
which is what gets the 16 SDMA engines close to line rate.
"""

import sys

sys.path.insert(0, "/opt/trn_rl_repo")

import numpy as np

import concourse.bass as bass
import concourse.mybir as mybir
import concourse.tile as tile
from concourse import bacc
from concourse.bass_utils import run_bass_kernel_spmd

B, S, M, N, D = 4, 4096, 4, 4, 2048
TOK = B * S  # 16384 tokens
N_CORES = 8
G = 25  # tokens per PE group (contraction K = 5*G = 125 <= 128)
KDIM = 5 * G  # 125
MF = N * G  # 100 output partitions per group
TPC = 2050  # padded tokens per core (= 82 * 25)
NG = TPC // G  # 82 groups per core
TOKP = TPC * N_CORES  # 16400 padded tokens total
DCH = 512  # moving free-dim chunk (one PSUM bank)

# Stashed BassKernelResults of the last kernel() call (for profiling in test
# harnesses).
LAST_RESULTS = None
LAST_IN_MAPS = None

# All bulk DMAs via gpsimd SWDGE (HWDGE concentrates reads on ~5 and writes
# on ~10 of the 16 SDMA engines; SWDGE spreads over all 16). Moderate chunks
# (gp=2 -> 8KB per-partition descriptors, which stream ~15% faster than 4KB
# ones) with deep double-buffering; output DMAs delayed a few chunks so they
# never head-of-line block input DMAs in the Pool FIFO behind a compute-done
# semaphore wait.
#
# wmode="onchip": the 25x-inflated block-diagonal W (2 MB) is never read from
# HBM; instead the compact caug (82 KB) is loaded once and each group's W is
# built on VectorE as mask * broadcast(caug_g) (the mask selects the
# block-diagonal positions, so the product equals the host-built W exactly).
#
# copy_banks: PSUM->SBUF evacuation copies handle this many PSUM banks per
# instruction; per-instruction fixed cost (~0.5us) otherwise makes
# VectorE/ScalarE the bottleneck below ~270us.
BUILD_KWARGS = dict(
    gp=2,
    abufs=8,
    obufs=7,
    in_eng="gpsimd",
    out_eng="gpsimd",
    out_delay=5,
    wmode="onchip",
    weng="sync",
    wsplit=2,
    copy_banks=2,
    mm_dtype="float16",
    out_dtype="float16",
)


def _chunks(gp, sched=None):
    """Chunk list [(g0, cgp)]. sched: explicit list of chunk sizes (must sum
    to NG), e.g. ramped [2,2,4,8,...,8,4,2,2] to shrink pipeline fill/drain
    while keeping big descriptors in steady state."""
    out = []
    g = 0
    if sched is not None:
        assert sum(sched) == NG, (sum(sched), NG)
        for c in sched:
            out.append((g, c))
            g += c
        return out
    while g < NG:
        out.append((g, min(gp, NG - g)))
        g += out[-1][1]
    return out


def _build_program(gp=2, abufs=8, obufs=7, in_eng="gpsimd", out_eng="gpsimd",
                   out_delay=5, wmode="onchip", weng="sync", wsplit=2,
                   mm_dtype="float16", out_dtype="float16", pbufs=None,
                   in_split=1, sched=None, copy_banks=2, wcbufs=6,
                   copy_pat="vs"):
    if pbufs is None:
        pbufs = 8 // copy_banks  # 8 PSUM banks total
    f32 = mybir.dt.float32
    mmdt = getattr(mybir.dt, mm_dtype)
    outdt = getattr(mybir.dt, out_dtype)
    nc = bacc.Bacc(None, target_bir_lowering=False)
    xa = nc.dram_tensor("xa", [KDIM, NG * D], mmdt, kind="ExternalInput")
    if wmode == "onchip":
        cw = nc.dram_tensor("cw", [KDIM, NG * N], mmdt, kind="ExternalInput")
        mk = nc.dram_tensor("msk", [KDIM, MF], mmdt, kind="ExternalInput")
    else:
        wb = nc.dram_tensor("wb", [KDIM, NG * MF], mmdt, kind="ExternalInput")
    y = nc.dram_tensor("y", [MF, NG * D], outdt, kind="ExternalOutput")

    chunks = _chunks(gp, sched)

    with tile.TileContext(nc) as tc:
        with (
            tc.tile_pool(name="wpool", bufs=1) as wpool,
            tc.tile_pool(name="wcpool", bufs=wcbufs) as wcpool,
            tc.tile_pool(name="apool", bufs=abufs) as apool,
            tc.tile_pool(name="opool", bufs=obufs) as opool,
            tc.tile_pool(name="psum", bufs=pbufs, space=bass.MemorySpace.PSUM) as psum,
        ):
            w_eng = getattr(nc, weng)
            if wmode == "onchip":
                cwt = wpool.tile([KDIM, NG * N], mmdt, tag="cw")
                mkt = wpool.tile([KDIM, MF], mmdt, tag="mk")
                w_eng.dma_start(cwt[:], cw[:])
                w_eng.dma_start(mkt[:], mk[:])
                mkt_v = mkt[:].rearrange("p (u n) -> p u n", u=G)

                def w_slice(g):
                    # W_g[p, 4u+n] = mask[p, 4u+n] * cw[p, 4g+n]  (broadcast
                    # over u) -- equals the block-diagonal W exactly since
                    # mask is 0/1 and fp16*1.0 is exact.
                    wt = wcpool.tile([KDIM, MF], mmdt, tag="wc")
                    src = cwt[:, g * N : (g + 1) * N].rearrange(
                        "p (u n) -> p u n", u=1
                    )
                    b0, b1 = bass.broadcast_tensor_aps(src, mkt_v)
                    nc.vector.tensor_mul(
                        wt[:].rearrange("p (u n) -> p u n", u=G), b1, b0
                    )
                    return wt[:]
            else:
                gper = (NG + wsplit - 1) // wsplit
                wt_tiles = []
                for wi in range(wsplit):
                    glo = wi * gper
                    ghi = min(NG, (wi + 1) * gper)
                    wtile = wpool.tile([KDIM, (ghi - glo) * MF], mmdt, tag=f"w{wi}")
                    w_eng.dma_start(wtile[:], wb[:, glo * MF : ghi * MF])
                    wt_tiles.append(wtile)

                def w_slice(g):
                    wi, off = divmod(g, gper)
                    return wt_tiles[wi][:, off * MF : (off + 1) * MF]

            nbk = D // DCH  # 4 PSUM-bank-sized pieces per group
            k = 0
            pending = []  # delayed output DMAs: (dst_ap, src_tile_ap)
            for ci, (g0, cgp) in enumerate(chunks):
                a = apool.tile([KDIM, cgp * D], mmdt, tag="a")
                src = xa[:, g0 * D : (g0 + cgp) * D]
                ie = getattr(nc, in_eng)
                if in_split == 1:
                    ie.dma_start(a[:], src)
                else:
                    step = (KDIM + in_split - 1) // in_split
                    for s0 in range(0, KDIM, step):
                        s1 = min(s0 + step, KDIM)
                        ie.dma_start(a[s0:s1], src[s0:s1])
                o = opool.tile([MF, cgp * D], outdt, tag="o")
                for gs in range(cgp):
                    g = g0 + gs
                    wsl = w_slice(g)
                    for dcb in range(0, nbk, copy_banks):
                        nb = min(copy_banks, nbk - dcb)
                        p = psum.tile([MF, nb * DCH], f32)
                        for j in range(nb):
                            dc = dcb + j
                            nc.tensor.matmul(
                                p[:, j * DCH : (j + 1) * DCH],
                                lhsT=wsl,
                                rhs=a[:, gs * D + dc * DCH : gs * D + (dc + 1) * DCH],
                                start=True,
                                stop=True,
                            )
                        dst = o[:, gs * D + dcb * DCH : gs * D + (dcb + nb) * DCH]
                        if copy_pat[k % len(copy_pat)] == "v":
                            nc.vector.tensor_copy(dst, p[:])
                        else:
                            nc.scalar.copy(dst, p[:])
                        k += 1
                y_dst = y[:, g0 * D : (g0 + cgp) * D]
                if out_eng == "gpsimd" and out_delay > 0:
                    pending.append((y_dst, o[:]))
                    if len(pending) > out_delay:
                        d0, s0 = pending.pop(0)
                        nc.gpsimd.dma_start(d0, s0)
                else:
                    getattr(nc, out_eng).dma_start(y_dst, o[:])
            for d0, s0 in pending:
                nc.gpsimd.dma_start(d0, s0)
    nc.compile()
    return nc


def kernel(x, residual, post, comb):
    global LAST_RESULTS, LAST_IN_MAPS
    x = np.asarray(x, dtype=np.float32)
    residual = np.asarray(residual, dtype=np.float32)
    post = np.asarray(post, dtype=np.float32)
    comb = np.asarray(comb, dtype=np.float32)

    mmdt = np.dtype(BUILD_KWARGS.get("mm_dtype", "float16"))

    # Host prepack: augmented data rows and block-diagonal weights, in fp16.
    # Padded tokens have zero weights -> zero output rows.
    xaug = np.zeros((TOKP, 5, D), mmdt)
    xaug[:TOK, 0, :] = x.reshape(TOK, D)
    xaug[:TOK, 1:, :] = residual.reshape(TOK, M, D)

    caug = np.zeros((TOKP, 5, N), mmdt)
    caug[:TOK, 0, :] = post.reshape(TOK, N)
    caug[:TOK, 1:, :] = comb.reshape(TOK, M, N)

    onchip = BUILD_KWARGS.get("wmode", "onchip") == "onchip"
    if onchip:
        # mask[5t+m', 4u+n] = (t == u); block-diag W is built on-device.
        t = np.arange(G)
        msk = (t[:, None, None, None] == t[None, None, :, None]).astype(mmdt)
        msk = np.ascontiguousarray(
            np.broadcast_to(msk, (G, 5, G, N)).reshape(KDIM, MF)
        )
    else:
        ngt = TOKP // G  # total groups
        wall = np.zeros((ngt, KDIM, MF), mmdt)
        t = np.arange(G)
        rows = np.broadcast_to(
            5 * t[:, None, None] + np.arange(5)[None, :, None], (G, 5, N)
        ).ravel()
        cols = np.broadcast_to(
            N * t[:, None, None] + np.arange(N)[None, None, :], (G, 5, N)
        ).ravel()
        wall[:, rows, cols] = caug.reshape(ngt, G * 5 * N)

    in_maps = []
    for c in range(N_CORES):
        # partition-major: xa[p = 5*t_loc + m', g*D + d]
        xa_c = np.ascontiguousarray(
            xaug[c * TPC : (c + 1) * TPC]
            .reshape(NG, G, 5, D)
            .transpose(1, 2, 0, 3)
            .reshape(KDIM, NG * D)
        )
        m = {"xa": xa_c}
        if onchip:
            # cw[p = 5t+m', g*N + n] = caug[token g*25+t, m', n]
            m["cw"] = np.ascontiguousarray(
                caug[c * TPC : (c + 1) * TPC]
                .reshape(NG, G, 5, N)
                .transpose(1, 2, 0, 3)
                .reshape(KDIM, NG * N)
            )
            m["msk"] = msk
        else:
            m["wb"] = np.ascontiguousarray(
                wall[c * NG : (c + 1) * NG].transpose(1, 0, 2).reshape(KDIM, NG * MF)
            )
        in_maps.append(m)

    LAST_IN_MAPS = in_maps
    nc = _build_program(**BUILD_KWARGS)
    res = None
    err = None
    for _attempt in range(3):  # transient NRT wedges clear on re-run
        try:
            res = run_bass_kernel_spmd(nc, in_maps, list(range(N_CORES)))
            break
        except Exception as e:  # noqa: BLE001
            err = e
    if res is None:
        raise err
    LAST_RESULTS = res

    # y_c[q = 4*t_loc + n, g*D + d] -> tokens
    y = np.concatenate(
        [
            res.results[c]["y"]
            .reshape(G, N, NG, D)
            .transpose(2, 0, 1, 3)
            .reshape(TPC, N, D)
            .astype(np.float32)
            for c in range(N_CORES)
        ],
        axis=0,
    )[:TOK]
    return np.ascontiguousarray(y.reshape(B, S, N, D))


# revision 24
# speedup vs baseline: 1.0690x; 1.0317x over previous
"""Trainium2 Bass kernel for nn_HcPost:

    out[b,s,n,d] = post[b,s,n] * x[b,s,d] + sum_m comb[b,s,m,n] * residual[b,s,m,d]

Strategy: per token this is a tiny K=5 contraction
    out[n,d] = sum_{m'} Caug[m',n] * Xaug[m',d]
with Xaug = [x; residual_0..3] and Caug = [post; comb_0..3].

We batch G=25 tokens into one TensorE matmul by building a block-diagonal
stationary weight matrix W[(t,m'), (t,n)] = Caug[t,m',n] (K=125, MF=100) on the
host, and streaming Xaug[(t,m'), d] as the moving operand. PSUM results
[(t,n), d] are evacuated to SBUF by VectorE/ScalarE and DMA'd out.

Sharding: tokens (B*S = 16384) split evenly across 8 NeuronCores (data
parallel, no cross-core communication). Tokens are padded to 2050/core so each
core runs 82 uniform groups of 25.

I/O layout: everything is fp16 (inputs converted on the host, output upcast on
the host; PE multiplies fp16 exactly into fp32 PSUM, so the only error is the
2^-11 input/output rounding, ~6e-4 rel vs the fp32 reference) and
PARTITION-MAJOR in HBM: xa[p, g*D+d] for p = 5*t_loc + m', y[q, g*D+d] for
q = 4*t_loc + n. That makes each DMA chunk of `gp` groups a single op whose
per-partition runs are gp*4KB contiguous, so SWDGE emits few, large
descriptors (vs one 4KB descriptor per partition in token-major layout),
which is what gets the 16 SDMA engines close to line rate.
"""

import sys

sys.path.insert(0, "/opt/trn_rl_repo")

import numpy as np

import concourse.bass as bass
import concourse.mybir as mybir
import concourse.tile as tile
from concourse import bacc
from concourse.bass_utils import run_bass_kernel_spmd

B, S, M, N, D = 4, 4096, 4, 4, 2048
TOK = B * S  # 16384 tokens
N_CORES = 8
G = 25  # tokens per PE group (contraction K = 5*G = 125 <= 128)
KDIM = 5 * G  # 125
MF = N * G  # 100 output partitions per group
TPC = 2050  # padded tokens per core (= 82 * 25)
NG = TPC // G  # 82 groups per core
TOKP = TPC * N_CORES  # 16400 padded tokens total
DCH = 512  # moving free-dim chunk (one PSUM bank)

# Stashed BassKernelResults of the last kernel() call (for profiling in test
# harnesses).
LAST_RESULTS = None
LAST_IN_MAPS = None

# All bulk DMAs via gpsimd SWDGE (HWDGE concentrates reads on ~5 and writes
# on ~10 of the 16 SDMA engines; SWDGE spreads over all 16). Moderate chunks
# (gp=2 -> 8KB per-partition descriptors, which stream ~15% faster than 4KB
# ones) with deep double-buffering; output DMAs delayed a few chunks so they
# never head-of-line block input DMAs in the Pool FIFO behind a compute-done
# semaphore wait.
#
# wmode="onchip": the 25x-inflated block-diagonal W (2 MB) is never read from
# HBM; instead the compact caug (82 KB) is loaded once and each group's W is
# built on VectorE as mask * broadcast(caug_g) (the mask selects the
# block-diagonal positions, so the product equals the host-built W exactly).
#
# copy_banks: PSUM->SBUF evacuation copies handle this many PSUM banks per
# instruction; per-instruction fixed cost (~0.5us) otherwise makes
# VectorE/ScalarE the bottleneck below ~270us.
BUILD_KWARGS = dict(
    gp=2,
    abufs=9,
    obufs=8,
    in_eng="gpsimd",
    out_eng="gpsimd",
    out_delay=5,
    wmode="onchip1",
    weng="sync",
    wsplit=2,
    copy_banks=2,
    mm_dtype="float16",
    out_dtype="float16",
)


def _chunks(gp, sched=None):
    """Chunk list [(g0, cgp)]. sched: explicit list of chunk sizes (must sum
    to NG), e.g. ramped [2,2,4,8,...,8,4,2,2] to shrink pipeline fill/drain
    while keeping big descriptors in steady state."""
    out = []
    g = 0
    if sched is not None:
        assert sum(sched) == NG, (sum(sched), NG)
        for c in sched:
            out.append((g, c))
            g += c
        return out
    while g < NG:
        out.append((g, min(gp, NG - g)))
        g += out[-1][1]
    return out


def _build_program(gp=2, abufs=8, obufs=7, in_eng="gpsimd", out_eng="gpsimd",
                   out_delay=5, wmode="onchip", weng="sync", wsplit=2,
                   mm_dtype="float16", out_dtype="float16", pbufs=None,
                   in_split=1, sched=None, copy_banks=2, wcbufs=6,
                   copy_pat="vs"):
    if pbufs is None:
        pbufs = 8 // copy_banks  # 8 PSUM banks total
    f32 = mybir.dt.float32
    mmdt = getattr(mybir.dt, mm_dtype)
    outdt = getattr(mybir.dt, out_dtype)
    nc = bacc.Bacc(None, target_bir_lowering=False)
    xa = nc.dram_tensor("xa", [KDIM, NG * D], mmdt, kind="ExternalInput")
    if wmode in ("onchip", "onchip1"):
        cw = nc.dram_tensor("cw", [KDIM, NG * N], mmdt, kind="ExternalInput")
        mk = nc.dram_tensor("msk", [KDIM, MF], mmdt, kind="ExternalInput")
    else:
        wb = nc.dram_tensor("wb", [KDIM, NG * MF], mmdt, kind="ExternalInput")
    y = nc.dram_tensor("y", [MF, NG * D], outdt, kind="ExternalOutput")

    chunks = _chunks(gp, sched)

    with tile.TileContext(nc) as tc:
        with (
            tc.tile_pool(name="wpool", bufs=1) as wpool,
            tc.tile_pool(name="wcpool", bufs=wcbufs) as wcpool,
            tc.tile_pool(name="apool", bufs=abufs) as apool,
            tc.tile_pool(name="opool", bufs=obufs) as opool,
            tc.tile_pool(name="psum", bufs=pbufs, space=bass.MemorySpace.PSUM) as psum,
        ):
            w_eng = getattr(nc, weng)
            if wmode == "onchip1":
                # Load compact caug (82 KB) + mask (25 KB) once, then build
                # the whole 2 MB block-diagonal W_all in ONE VectorE op via
                # double-broadcast: W[p, g,u,n] = msk[p, u,n] * cw[p, g,n].
                cwt = wpool.tile([KDIM, NG * N], mmdt, tag="cw")
                mkt = wpool.tile([KDIM, MF], mmdt, tag="mk")
                w_eng.dma_start(cwt[:], cw[:])
                w_eng.dma_start(mkt[:], mk[:])
                wall_t = wpool.tile([KDIM, NG * MF], mmdt, tag="wall")
                b_mk, b_cw = bass.broadcast_tensor_aps(
                    mkt[:].rearrange("p (g u n) -> p g u n", g=1, u=G),
                    cwt[:].rearrange("p (g u n) -> p g u n", g=NG, u=1),
                )
                nc.vector.tensor_mul(
                    wall_t[:].rearrange("p (g u n) -> p g u n", g=NG, u=G),
                    b_mk,
                    b_cw,
                )

                def w_slice(g):
                    return wall_t[:, g * MF : (g + 1) * MF]
            elif wmode == "onchip":
                cwt = wpool.tile([KDIM, NG * N], mmdt, tag="cw")
                mkt = wpool.tile([KDIM, MF], mmdt, tag="mk")
                w_eng.dma_start(cwt[:], cw[:])
                w_eng.dma_start(mkt[:], mk[:])
                mkt_v = mkt[:].rearrange("p (u n) -> p u n", u=G)

                def w_slice(g):
                    # W_g[p, 4u+n] = mask[p, 4u+n] * cw[p, 4g+n]  (broadcast
                    # over u) -- equals the block-diagonal W exactly since
                    # mask is 0/1 and fp16*1.0 is exact.
                    wt = wcpool.tile([KDIM, MF], mmdt, tag="wc")
                    src = cwt[:, g * N : (g + 1) * N].rearrange(
                        "p (u n) -> p u n", u=1
                    )
                    b0, b1 = bass.broadcast_tensor_aps(src, mkt_v)
                    nc.vector.tensor_mul(
                        wt[:].rearrange("p (u n) -> p u n", u=G), b1, b0
                    )
                    return wt[:]
            else:
                gper = (NG + wsplit - 1) // wsplit
                wt_tiles = []
                for wi in range(wsplit):
                    glo = wi * gper
                    ghi = min(NG, (wi + 1) * gper)
                    wtile = wpool.tile([KDIM, (ghi - glo) * MF], mmdt, tag=f"w{wi}")
                    w_eng.dma_start(wtile[:], wb[:, glo * MF : ghi * MF])
                    wt_tiles.append(wtile)

                def w_slice(g):
                    wi, off = divmod(g, gper)
                    return wt_tiles[wi][:, off * MF : (off + 1) * MF]

            nbk = D // DCH  # 4 PSUM-bank-sized pieces per group
            k = 0
            pending = []  # delayed output DMAs: (dst_ap, src_tile_ap)
            for ci, (g0, cgp) in enumerate(chunks):
                a = apool.tile([KDIM, cgp * D], mmdt, tag="a")
                src = xa[:, g0 * D : (g0 + cgp) * D]
                ie = getattr(nc, in_eng)
                if in_split == 1:
                    ie.dma_start(a[:], src)
                else:
                    step = (KDIM + in_split - 1) // in_split
                    for s0 in range(0, KDIM, step):
                        s1 = min(s0 + step, KDIM)
                        ie.dma_start(a[s0:s1], src[s0:s1])
                o = opool.tile([MF, cgp * D], outdt, tag="o")
                for gs in range(cgp):
                    g = g0 + gs
                    wsl = w_slice(g)
                    for dcb in range(0, nbk, copy_banks):
                        nb = min(copy_banks, nbk - dcb)
                        p = psum.tile([MF, nb * DCH], f32)
                        for j in range(nb):
                            dc = dcb + j
                            nc.tensor.matmul(
                                p[:, j * DCH : (j + 1) * DCH],
                                lhsT=wsl,
                                rhs=a[:, gs * D + dc * DCH : gs * D + (dc + 1) * DCH],
                                start=True,
                                stop=True,
                            )
                        dst = o[:, gs * D + dcb * DCH : gs * D + (dcb + nb) * DCH]
                        if copy_pat[k % len(copy_pat)] == "v":
                            nc.vector.tensor_copy(dst, p[:])
                        else:
                            nc.scalar.copy(dst, p[:])
                        k += 1
                y_dst = y[:, g0 * D : (g0 + cgp) * D]
                if out_eng == "gpsimd" and out_delay > 0:
                    pending.append((y_dst, o[:]))
                    if len(pending) > out_delay:
                        d0, s0 = pending.pop(0)
                        nc.gpsimd.dma_start(d0, s0)
                else:
                    getattr(nc, out_eng).dma_start(y_dst, o[:])
            for d0, s0 in pending:
                nc.gpsimd.dma_start(d0, s0)
    nc.compile()
    return nc


def kernel(x, residual, post, comb):
    global LAST_RESULTS, LAST_IN_MAPS
    x = np.asarray(x, dtype=np.float32)
    residual = np.asarray(residual, dtype=np.float32)
    post = np.asarray(post, dtype=np.float32)
    comb = np.asarray(comb, dtype=np.float32)

    mmdt = np.dtype(BUILD_KWARGS.get("mm_dtype", "float16"))

    # Host prepack: augmented data rows and block-diagonal weights, in fp16.
    # Padded tokens have zero weights -> zero output rows.
    xaug = np.zeros((TOKP, 5, D), mmdt)
    xaug[:TOK, 0, :] = x.reshape(TOK, D)
    xaug[:TOK, 1:, :] = residual.reshape(TOK, M, D)

    caug = np.zeros((TOKP, 5, N), mmdt)
    caug[:TOK, 0, :] = post.reshape(TOK, N)
    caug[:TOK, 1:, :] = comb.reshape(TOK, M, N)

    onchip = BUILD_KWARGS.get("wmode", "onchip") in ("onchip", "onchip1")
    if onchip:
        # mask[5t+m', 4u+n] = (t == u); block-diag W is built on-device.
        t = np.arange(G)
        msk = (t[:, None, None, None] == t[None, None, :, None]).astype(mmdt)
        msk = np.ascontiguousarray(
            np.broadcast_to(msk, (G, 5, G, N)).reshape(KDIM, MF)
        )
    else:
        ngt = TOKP // G  # total groups
        wall = np.zeros((ngt, KDIM, MF), mmdt)
        t = np.arange(G)
        rows = np.broadcast_to(
            5 * t[:, None, None] + np.arange(5)[None, :, None], (G, 5, N)
        ).ravel()
        cols = np.broadcast_to(
            N * t[:, None, None] + np.arange(N)[None, None, :], (G, 5, N)
        ).ravel()
        wall[:, rows, cols] = caug.reshape(ngt, G * 5 * N)

    in_maps = []
    for c in range(N_CORES):
        # partition-major: xa[p = 5*t_loc + m', g*D + d]
        xa_c = np.ascontiguousarray(
            xaug[c * TPC : (c + 1) * TPC]
            .reshape(NG, G, 5, D)
            .transpose(1, 2, 0, 3)
            .reshape(KDIM, NG * D)
        )
        m = {"xa": xa_c}
        if onchip:
            # cw[p = 5t+m', g*N + n] = caug[token g*25+t, m', n]
            m["cw"] = np.ascontiguousarray(
                caug[c * TPC : (c + 1) * TPC]
                .reshape(NG, G, 5, N)
                .transpose(1, 2, 0, 3)
                .reshape(KDIM, NG * N)
            )
            m["msk"] = msk
        else:
            m["wb"] = np.ascontiguousarray(
                wall[c * NG : (c + 1) * NG].transpose(1, 0, 2).reshape(KDIM, NG * MF)
            )
        in_maps.append(m)

    LAST_IN_MAPS = in_maps
    nc = _build_program(**BUILD_KWARGS)
    res = None
    err = None
    for _attempt in range(3):  # transient NRT wedges clear on re-run
        try:
            res = run_bass_kernel_spmd(nc, in_maps, list(range(N_CORES)))
            break
        except Exception as e:  # noqa: BLE001
            err = e
    if res is None:
        raise err
    LAST_RESULTS = res

    # y_c[q = 4*t_loc + n, g*D + d] -> tokens
    y = np.concatenate(
        [
            res.results[c]["y"]
            .reshape(G, N, NG, D)
            .transpose(2, 0, 1, 3)
            .reshape(TPC, N, D)
            .astype(np.float32)
            for c in range(N_CORES)
        ],
        axis=0,
    )[:TOK]
    return np.ascontiguousarray(y.reshape(B, S, N, D))
